# revision 28
# baseline (speedup 1.0000x reference)
"""L1 loss (mean |yhat - y|) over (64, 128, 4096) fp32 tensors on 8 TRN2 cores.

Strategy: pure data-parallel over the batch dim; core i takes batch rows
[8i, 8i+8), i.e. 4,194,304 elements per input tensor. The host casts
everything to fp8 e4m3 (rel-err budget is 2e-2; measured end-to-end
error ~7e-4), minimizing HBM traffic (8 MiB/core), and splits each
core's elements into two regions:

 - Tensor-engine region (11 tiles of [128, 4096] plus 2 cooldown tiles
   of [128, 2048], fp8): each tile stacks 64 rows of yhat on partitions
   0:64 and the matching 64 rows of y on partitions 64:128. A constant
   [128, 64] +/-identity stationary makes the PE array compute
   out[m,f] = z[m,f] - z[64+m,f], so the *subtraction* runs at matmul
   rates (~230ns per 512 columns) into PSUM. Consecutive 512-column
   chunks land at PSUM partition offsets 0 and 64, so each tile fills a
   [128, cols/2] fp32 PSUM block (two blocks ping-pong across PSUM).
   The diffs are exact in fp32. The first two DMAs are tensor-engine
   tiles so this longest stream starts as early as possible; the last
   two tiles are half-size so the post-DMA drain is short.
 - Vector-engine region (2 interleaved [yhat|y] tiles of 4096 cols):
   classic tensor_sub on the DVE (fp8 at 1x), DMA'd mid-stream.

The abs+sum reduction of each PSUM block / d tile is split between the
scalar engine (activation(Abs, accum_out=...), 0.833 ns/col) and the
vector engine (tensor_reduce(add, apply_absolute_value), 1.04 ns/col),
alternating block consumers so both drain PSUM in parallel. Streams:
DMA ~24us, TensorE ~24us, ACT ~22us, DVE ~21us. (The previous best
non-matmul variant was stuck at 30-32us streams: the DVE subtract was
the wall, since every DVE op with an accumulator output runs at 1x on
HW and fp8 operands disable the 2x/4x packed modes.)

Partials land in fp32 columns of a [128, 15] accumulator; the host
combines them in float64 and divides by the global element count.
"""

import numpy as np
import ml_dtypes

import concourse.bacc as bacc
import concourse.mybir as mybir
import concourse.tile as tile
from concourse.bass_utils import run_bass_kernel_spmd

N_CORES = 8
FULL_SHAPE = (64, 128, 4096)
TOTAL_ELEMS = FULL_SHAPE[0] * FULL_SHAPE[1] * FULL_SHAPE[2]  # 33,554,432
ELEMS_PER_CORE = TOTAL_ELEMS // N_CORES                      # 4,194,304

P = 128
MM_N = 512                                # matmul moving free-dim limit
T_TILE_COLS = [4096] * 11 + [2048, 2048]  # 64-row-space cols per T tile
T_ELEMS = 64 * sum(T_TILE_COLS)           # 3,145,728 elems per tensor
D_TILE_COLS = [4096, 4096]                # 128-row-space cols per D tile
assert T_ELEMS + P * sum(D_TILE_COLS) == ELEMS_PER_CORE

N_T = len(T_TILE_COLS)
N_D = len(D_TILE_COLS)
# DMA order: w, T0, T1, D0, T2..T4, D1, T5..T12 — T first so the
# tensor-engine stream starts early; D tiles arrive when the DVE needs
# them; small T tiles last for a short drain.
DMA_ORDER = (
    [("t", 0), ("t", 1), ("d", 0), ("t", 2), ("t", 3), ("t", 4), ("d", 1)]
    + [("t", j) for j in range(5, N_T)]
)
DVE_BLOCKS = {1, 3, 5, 7, 9, 11}          # PSUM blocks reduced on the DVE

_nc_cache = []


def _build_nc():
    nc = bacc.Bacc("TRN2", target_bir_lowering=False, debug=False)
    w = nc.declare_dram_parameter("w", [P, 64], mybir.dt.float8e4, isOutput=False)
    zt_dram = [
        nc.declare_dram_parameter(f"zt{j}", [P, c], mybir.dt.float8e4, isOutput=False)
        for j, c in enumerate(T_TILE_COLS)
    ]
    zd_dram = [
        nc.declare_dram_parameter(
            f"zd{i}", [P, 2 * c], mybir.dt.float8e4, isOutput=False
        )
        for i, c in enumerate(D_TILE_COLS)
    ]
    n_acc = N_T + N_D
    out = nc.declare_dram_parameter("out", [P, n_acc], mybir.dt.float32, isOutput=True)

    with tile.TileContext(nc) as tc:
        with (
            tc.tile_pool(name="io", bufs=1) as io_pool,
            tc.tile_pool(name="wk", bufs=2) as wk_pool,
            tc.psum_pool(name="ps", bufs=2) as ps_pool,
            tc.tile_pool(name="acc", bufs=1) as acc_pool,
        ):
            acc = acc_pool.tile([P, n_acc], mybir.dt.float32)
            wt = io_pool.tile([P, 64], mybir.dt.float8e4, tag="w")
            nc.sync.dma_start(wt[:], w[:])
            zts = {}
            zds = {}
            for kind, idx in DMA_ORDER:
                if kind == "t":
                    c = T_TILE_COLS[idx]
                    zt = io_pool.tile([P, c], mybir.dt.float8e4, tag=f"zt{idx}")
                    nc.sync.dma_start(zt[:], zt_dram[idx][:])
                    zts[idx] = zt
                else:
                    c = D_TILE_COLS[idx]
                    zd = io_pool.tile([P, 2 * c], mybir.dt.float8e4, tag=f"zd{idx}")
                    nc.sync.dma_start(zd[:], zd_dram[idx][:])
                    zds[idx] = zd

            def emit_d_tile(i):
                c = D_TILE_COLS[i]
                dd = wk_pool.tile([P, c], mybir.dt.bfloat16, tag="d")
                nc.vector.tensor_sub(dd[:], zds[i][:, 0:c], zds[i][:, c : 2 * c])
                a = wk_pool.tile([P, c], mybir.dt.bfloat16, tag=f"da{i}")
                nc.scalar.activation(
                    a[:],
                    dd[:],
                    mybir.ActivationFunctionType.Abs,
                    accum_out=acc[:, N_T + i : N_T + i + 1],
                )

            for j, c in enumerate(T_TILE_COLS):
                blk = ps_pool.tile([P, c // 2], mybir.dt.float32, tag="blk")
                for s in range(0, c, MM_N):
                    half = (s // MM_N) % 2
                    c0 = (s // (2 * MM_N)) * MM_N
                    nc.tensor.matmul(
                        blk[64 * half : 64 * half + 64, c0 : c0 + MM_N],
                        wt[:],
                        zts[j][:, s : s + MM_N],
                        start=True,
                        stop=True,
                    )
                if j in DVE_BLOCKS:
                    nc.vector.tensor_reduce(
                        acc[:, j : j + 1],
                        blk[:],
                        axis=mybir.AxisListType.X,
                        op=mybir.AluOpType.add,
                        apply_absolute_value=True,
                    )
                else:
                    ab = wk_pool.tile([P, c // 2], mybir.dt.bfloat16, tag="ab")
                    nc.scalar.activation(
                        ab[:],
                        blk[:],
                        mybir.ActivationFunctionType.Abs,
                        accum_out=acc[:, j : j + 1],
                    )
                # D tiles are consumed between T blocks, matching DMA arrival.
                if j == 2:
                    emit_d_tile(0)
                if j == 6:
                    emit_d_tile(1)
            nc.sync.dma_start(out[:], acc[:])
    nc.compile()
    return nc


def _get_nc():
    if not _nc_cache:
        _nc_cache.append(_build_nc())
    return _nc_cache[0]


def _shard_inputs(yhat: np.ndarray, y: np.ndarray) -> list[dict[str, np.ndarray]]:
    yh = np.ascontiguousarray(yhat, dtype=np.float32).reshape(N_CORES, ELEMS_PER_CORE)
    yv = np.ascontiguousarray(y, dtype=np.float32).reshape(N_CORES, ELEMS_PER_CORE)
    f8 = ml_dtypes.float8_e4m3

    wm = np.zeros((P, 64), dtype=np.float32)
    for m in range(64):
        wm[m, m] = 1.0
        wm[m + 64, m] = -1.0
    wm = wm.astype(f8)

    t_cols_total = sum(T_TILE_COLS)
    in_maps = []
    for c in range(N_CORES):
        m = {"w": wm}
        yh_t = yh[c, :T_ELEMS].reshape(64, t_cols_total).astype(f8)
        yv_t = yv[c, :T_ELEMS].reshape(64, t_cols_total).astype(f8)
        off = 0
        for j, tc_ in enumerate(T_TILE_COLS):
            zt = np.empty((P, tc_), dtype=f8)
            zt[0:64] = yh_t[:, off : off + tc_]
            zt[64:128] = yv_t[:, off : off + tc_]
            m[f"zt{j}"] = zt
            off += tc_
        d_cols_total = sum(D_TILE_COLS)
        yh_d = yh[c, T_ELEMS:].reshape(P, d_cols_total)
        yv_d = yv[c, T_ELEMS:].reshape(P, d_cols_total)
        off = 0
        for i, dc in enumerate(D_TILE_COLS):
            zd = np.empty((P, 2 * dc), dtype=f8)
            zd[:, 0:dc] = yh_d[:, off : off + dc]
            zd[:, dc : 2 * dc] = yv_d[:, off : off + dc]
            m[f"zd{i}"] = zd
            off += dc
        in_maps.append(m)
    return in_maps


def kernel(yhat: np.ndarray, y: np.ndarray) -> np.ndarray:
    nc = _get_nc()
    in_maps = _shard_inputs(yhat, y)
    res = run_bass_kernel_spmd(nc, in_maps, list(range(N_CORES)))
    total = np.float64(0.0)
    for r in res.results:
        total += r["out"].astype(np.float64).sum()
    return np.asarray(total / TOTAL_ELEMS, dtype=np.float32)


# revision 29
# speedup vs baseline: 1.1139x; 1.1139x over previous
"""L1 loss (mean |yhat - y|) over (64, 128, 4096) fp32 tensors on 8 TRN2 cores.

Strategy: pure data-parallel over the batch dim; core i takes batch rows
[8i, 8i+8), i.e. 4,194,304 elements per input tensor. The host casts
everything to fp8 e4m3 (rel-err budget is 2e-2; measured end-to-end
error ~7e-4), minimizing HBM traffic (8 MiB/core), and splits each
core's elements into two regions:

 - Tensor-engine region (11 tiles of [128, 4096] plus 2 cooldown tiles
   of [128, 2048], fp8): each tile stacks 64 rows of yhat on partitions
   0:64 and the matching 64 rows of y on partitions 64:128. A constant
   [128, 64] +/-identity stationary makes the PE array compute
   out[m,f] = z[m,f] - z[64+m,f], so the *subtraction* runs at matmul
   rates (~230ns per 512 columns) into PSUM. Consecutive 512-column
   chunks land at PSUM partition offsets 0 and 64, so each tile fills a
   [128, cols/2] fp32 PSUM block (two blocks ping-pong across PSUM).
   The diffs are exact in fp32. The first two DMAs are tensor-engine
   tiles so this longest stream starts as early as possible; the last
   two tiles are half-size so the post-DMA drain is short.
 - Vector-engine region (2 interleaved [yhat|y] tiles of 4096 cols):
   classic tensor_sub on the DVE (fp8 at 1x), DMA'd mid-stream.

The abs+sum reduction of each PSUM block / d tile is split between the
scalar engine (activation(Abs, accum_out=...), 0.833 ns/col) and the
vector engine (tensor_reduce(add, apply_absolute_value), 1.04 ns/col),
alternating block consumers so both drain PSUM in parallel. Streams:
DMA ~24us, TensorE ~24us, ACT ~22us, DVE ~21us. (The previous best
non-matmul variant was stuck at 30-32us streams: the DVE subtract was
the wall, since every DVE op with an accumulator output runs at 1x on
HW and fp8 operands disable the 2x/4x packed modes.)

Partials land in fp32 columns of a [128, 15] accumulator; the host
combines them in float64 and divides by the global element count.
"""

import numpy as np
import ml_dtypes

import concourse.bacc as bacc
import concourse.mybir as mybir
import concourse.tile as tile
from concourse.bass_utils import run_bass_kernel_spmd

N_CORES = 8
FULL_SHAPE = (64, 128, 4096)
TOTAL_ELEMS = FULL_SHAPE[0] * FULL_SHAPE[1] * FULL_SHAPE[2]  # 33,554,432
ELEMS_PER_CORE = TOTAL_ELEMS // N_CORES                      # 4,194,304

P = 128
MM_N = 512                                # matmul moving free-dim limit
T_TILE_COLS = [2048, 2048] + [4096] * 10 + [2048, 2048]  # 64-row-space cols
T_ELEMS = 64 * sum(T_TILE_COLS)           # 3,145,728 elems per tensor
D_TILE_COLS = [4096, 4096]                # 128-row-space cols per D tile
assert T_ELEMS + P * sum(D_TILE_COLS) == ELEMS_PER_CORE

N_T = len(T_TILE_COLS)
N_D = len(D_TILE_COLS)
# DMA order: w, T0, T1, D0, T2..T4, D1, T5..T12 — T first so the
# tensor-engine stream starts early; D tiles arrive when the DVE needs
# them; small T tiles last for a short drain.
DMA_ORDER = (
    [("t", 0), ("t", 1), ("t", 2), ("d", 0), ("t", 3), ("t", 4), ("d", 1)]
    + [("t", j) for j in range(5, N_T)]
)
DVE_BLOCKS = {1, 3, 5, 7, 9, 11}          # PSUM blocks reduced on the DVE

_nc_cache = []


def _build_nc():
    nc = bacc.Bacc("TRN2", target_bir_lowering=False, debug=False)
    w = nc.declare_dram_parameter("w", [P, 64], mybir.dt.float8e4, isOutput=False)
    zt_dram = [
        nc.declare_dram_parameter(f"zt{j}", [P, c], mybir.dt.float8e4, isOutput=False)
        for j, c in enumerate(T_TILE_COLS)
    ]
    zd_dram = [
        nc.declare_dram_parameter(
            f"zd{i}", [P, 2 * c], mybir.dt.float8e4, isOutput=False
        )
        for i, c in enumerate(D_TILE_COLS)
    ]
    n_acc = N_T + N_D
    out = nc.declare_dram_parameter("out", [P, n_acc], mybir.dt.float32, isOutput=True)

    with tile.TileContext(nc) as tc:
        with (
            tc.tile_pool(name="io", bufs=1) as io_pool,
            tc.tile_pool(name="wk", bufs=2) as wk_pool,
            tc.psum_pool(name="ps", bufs=2) as ps_pool,
            tc.tile_pool(name="acc", bufs=1) as acc_pool,
        ):
            acc = acc_pool.tile([P, n_acc], mybir.dt.float32)
            wt = io_pool.tile([P, 64], mybir.dt.float8e4, tag="w")
            nc.sync.dma_start(wt[:], w[:])
            zts = {}
            zds = {}
            for kind, idx in DMA_ORDER:
                if kind == "t":
                    c = T_TILE_COLS[idx]
                    zt = io_pool.tile([P, c], mybir.dt.float8e4, tag=f"zt{idx}")
                    nc.sync.dma_start(zt[:], zt_dram[idx][:])
                    zts[idx] = zt
                else:
                    c = D_TILE_COLS[idx]
                    zd = io_pool.tile([P, 2 * c], mybir.dt.float8e4, tag=f"zd{idx}")
                    nc.sync.dma_start(zd[:], zd_dram[idx][:])
                    zds[idx] = zd

            def emit_d_tile(i):
                c = D_TILE_COLS[i]
                dd = wk_pool.tile([P, c], mybir.dt.bfloat16, tag="d")
                nc.vector.tensor_sub(dd[:], zds[i][:, 0:c], zds[i][:, c : 2 * c])
                a = wk_pool.tile([P, c], mybir.dt.bfloat16, tag=f"da{i}")
                nc.scalar.activation(
                    a[:],
                    dd[:],
                    mybir.ActivationFunctionType.Abs,
                    accum_out=acc[:, N_T + i : N_T + i + 1],
                )

            for j, c in enumerate(T_TILE_COLS):
                blk = ps_pool.tile([P, c // 2], mybir.dt.float32, tag="blk")
                for s in range(0, c, MM_N):
                    half = (s // MM_N) % 2
                    c0 = (s // (2 * MM_N)) * MM_N
                    nc.tensor.matmul(
                        blk[64 * half : 64 * half + 64, c0 : c0 + MM_N],
                        wt[:],
                        zts[j][:, s : s + MM_N],
                        start=True,
                        stop=True,
                    )
                if j in DVE_BLOCKS:
                    nc.vector.tensor_reduce(
                        acc[:, j : j + 1],
                        blk[:],
                        axis=mybir.AxisListType.X,
                        op=mybir.AluOpType.add,
                        apply_absolute_value=True,
                    )
                else:
                    ab = wk_pool.tile([P, c // 2], mybir.dt.bfloat16, tag="ab")
                    nc.scalar.activation(
                        ab[:],
                        blk[:],
                        mybir.ActivationFunctionType.Abs,
                        accum_out=acc[:, j : j + 1],
                    )
                # D tiles are consumed between T blocks, matching DMA arrival.
                if j == 3:
                    emit_d_tile(0)
                if j == 6:
                    emit_d_tile(1)
            nc.sync.dma_start(out[:], acc[:])
    nc.compile()
    return nc


def _get_nc():
    if not _nc_cache:
        _nc_cache.append(_build_nc())
    return _nc_cache[0]


def _shard_inputs(yhat: np.ndarray, y: np.ndarray) -> list[dict[str, np.ndarray]]:
    yh = np.ascontiguousarray(yhat, dtype=np.float32).reshape(N_CORES, ELEMS_PER_CORE)
    yv = np.ascontiguousarray(y, dtype=np.float32).reshape(N_CORES, ELEMS_PER_CORE)
    f8 = ml_dtypes.float8_e4m3

    wm = np.zeros((P, 64), dtype=np.float32)
    for m in range(64):
        wm[m, m] = 1.0
        wm[m + 64, m] = -1.0
    wm = wm.astype(f8)

    t_cols_total = sum(T_TILE_COLS)
    in_maps = []
    for c in range(N_CORES):
        m = {"w": wm}
        yh_t = yh[c, :T_ELEMS].reshape(64, t_cols_total).astype(f8)
        yv_t = yv[c, :T_ELEMS].reshape(64, t_cols_total).astype(f8)
        off = 0
        for j, tc_ in enumerate(T_TILE_COLS):
            zt = np.empty((P, tc_), dtype=f8)
            zt[0:64] = yh_t[:, off : off + tc_]
            zt[64:128] = yv_t[:, off : off + tc_]
            m[f"zt{j}"] = zt
            off += tc_
        d_cols_total = sum(D_TILE_COLS)
        yh_d = yh[c, T_ELEMS:].reshape(P, d_cols_total)
        yv_d = yv[c, T_ELEMS:].reshape(P, d_cols_total)
        off = 0
        for i, dc in enumerate(D_TILE_COLS):
            zd = np.empty((P, 2 * dc), dtype=f8)
            zd[:, 0:dc] = yh_d[:, off : off + dc]
            zd[:, dc : 2 * dc] = yv_d[:, off : off + dc]
            m[f"zd{i}"] = zd
            off += dc
        in_maps.append(m)
    return in_maps


def kernel(yhat: np.ndarray, y: np.ndarray) -> np.ndarray:
    nc = _get_nc()
    in_maps = _shard_inputs(yhat, y)
    res = run_bass_kernel_spmd(nc, in_maps, list(range(N_CORES)))
    total = np.float64(0.0)
    for r in res.results:
        total += r["out"].astype(np.float64).sum()
    return np.asarray(total / TOTAL_ELEMS, dtype=np.float32)


# revision 30
# speedup vs baseline: 1.1256x; 1.0105x over previous
"""L1 loss (mean |yhat - y|) over (64, 128, 4096) fp32 tensors on 8 TRN2 cores.

Strategy: pure data-parallel over the batch dim; core i takes batch rows
[8i, 8i+8), i.e. 4,194,304 elements per input tensor. The host casts
everything to fp8 e4m3 (rel-err budget is 2e-2; measured end-to-end
error ~7e-4), minimizing HBM traffic (8 MiB/core), and splits each
core's elements into two regions:

 - Tensor-engine region (11 tiles of [128, 4096] plus 2 cooldown tiles
   of [128, 2048], fp8): each tile stacks 64 rows of yhat on partitions
   0:64 and the matching 64 rows of y on partitions 64:128. A constant
   [128, 64] +/-identity stationary makes the PE array compute
   out[m,f] = z[m,f] - z[64+m,f], so the *subtraction* runs at matmul
   rates (~230ns per 512 columns) into PSUM. Consecutive 512-column
   chunks land at PSUM partition offsets 0 and 64, so each tile fills a
   [128, cols/2] fp32 PSUM block (two blocks ping-pong across PSUM).
   The diffs are exact in fp32. The first two DMAs are tensor-engine
   tiles so this longest stream starts as early as possible; the last
   two tiles are half-size so the post-DMA drain is short.
 - Vector-engine region (2 interleaved [yhat|y] tiles of 4096 cols):
   classic tensor_sub on the DVE (fp8 at 1x), DMA'd mid-stream.

The abs+sum reduction of each PSUM block / d tile is split between the
scalar engine (activation(Abs, accum_out=...), 0.833 ns/col) and the
vector engine (tensor_reduce(add, apply_absolute_value), 1.04 ns/col),
alternating block consumers so both drain PSUM in parallel. Streams:
DMA ~24us, TensorE ~24us, ACT ~22us, DVE ~21us. (The previous best
non-matmul variant was stuck at 30-32us streams: the DVE subtract was
the wall, since every DVE op with an accumulator output runs at 1x on
HW and fp8 operands disable the 2x/4x packed modes.)

Partials land in fp32 columns of a [128, 15] accumulator; the host
combines them in float64 and divides by the global element count.
"""

import numpy as np
import ml_dtypes

import concourse.bacc as bacc
import concourse.mybir as mybir
import concourse.tile as tile
from concourse.bass_utils import run_bass_kernel_spmd

N_CORES = 8
FULL_SHAPE = (64, 128, 4096)
TOTAL_ELEMS = FULL_SHAPE[0] * FULL_SHAPE[1] * FULL_SHAPE[2]  # 33,554,432
ELEMS_PER_CORE = TOTAL_ELEMS // N_CORES                      # 4,194,304

P = 128
MM_N = 512                                # matmul moving free-dim limit
T_TILE_COLS = [4096] * 11 + [2048, 2048]  # 64-row-space cols per T tile
T_ELEMS = 64 * sum(T_TILE_COLS)           # 3,145,728 elems per tensor
D_TILE_COLS = [4096, 4096]                # 128-row-space cols per D tile
assert T_ELEMS + P * sum(D_TILE_COLS) == ELEMS_PER_CORE

N_T = len(T_TILE_COLS)
N_D = len(D_TILE_COLS)
# DMA order: w, T0, T1, D0, T2..T4, D1, T5..T12 — T first so the
# tensor-engine stream starts early; D tiles arrive when the DVE needs
# them; small T tiles last for a short drain.
DMA_ORDER = (
    [("t", 0), ("t", 1), ("d", 0), ("t", 2), ("t", 3), ("t", 4), ("d", 1)]
    + [("t", j) for j in range(5, N_T)]
)
DVE_BLOCKS = {1, 3, 5, 7, 9, 11}          # PSUM blocks reduced on the DVE

_nc_cache = []


def _build_nc():
    nc = bacc.Bacc("TRN2", target_bir_lowering=False, debug=False)
    w = nc.declare_dram_parameter("w", [P, 64], mybir.dt.float8e4, isOutput=False)
    zt_dram = [
        nc.declare_dram_parameter(f"zt{j}", [P, c], mybir.dt.float8e4, isOutput=False)
        for j, c in enumerate(T_TILE_COLS)
    ]
    zd_dram = [
        nc.declare_dram_parameter(
            f"zd{i}", [P, 2 * c], mybir.dt.float8e4, isOutput=False
        )
        for i, c in enumerate(D_TILE_COLS)
    ]
    n_acc = N_T + N_D
    out = nc.declare_dram_parameter("out", [P, n_acc], mybir.dt.float32, isOutput=True)

    with tile.TileContext(nc) as tc:
        with (
            tc.tile_pool(name="io", bufs=1) as io_pool,
            tc.tile_pool(name="wk", bufs=2) as wk_pool,
            tc.psum_pool(name="ps", bufs=2) as ps_pool,
            tc.tile_pool(name="acc", bufs=1) as acc_pool,
        ):
            acc = acc_pool.tile([P, n_acc], mybir.dt.float32)
            wt = io_pool.tile([P, 64], mybir.dt.float8e4, tag="w")
            nc.sync.dma_start(wt[:], w[:])
            zts = {}
            zds = {}
            for kind, idx in DMA_ORDER:
                if kind == "t":
                    c = T_TILE_COLS[idx]
                    zt = io_pool.tile([P, c], mybir.dt.float8e4, tag=f"zt{idx}")
                    nc.sync.dma_start(zt[:], zt_dram[idx][:])
                    zts[idx] = zt
                else:
                    c = D_TILE_COLS[idx]
                    zd = io_pool.tile([P, 2 * c], mybir.dt.float8e4, tag=f"zd{idx}")
                    nc.sync.dma_start(zd[:], zd_dram[idx][:])
                    zds[idx] = zd

            def emit_d_tile(i):
                c = D_TILE_COLS[i]
                dd = wk_pool.tile([P, c], mybir.dt.bfloat16, tag="d")
                nc.vector.tensor_sub(dd[:], zds[i][:, 0:c], zds[i][:, c : 2 * c])
                a = wk_pool.tile([P, c], mybir.dt.bfloat16, tag=f"da{i}")
                nc.scalar.activation(
                    a[:],
                    dd[:],
                    mybir.ActivationFunctionType.Abs,
                    accum_out=acc[:, N_T + i : N_T + i + 1],
                )

            for j, c in enumerate(T_TILE_COLS):
                blk = ps_pool.tile([P, c // 2], mybir.dt.float32, tag="blk")
                for s in range(0, c, MM_N):
                    half = (s // MM_N) % 2
                    c0 = (s // (2 * MM_N)) * MM_N
                    nc.tensor.matmul(
                        blk[64 * half : 64 * half + 64, c0 : c0 + MM_N],
                        wt[:],
                        zts[j][:, s : s + MM_N],
                        start=True,
                        stop=True,
                    )
                if j in DVE_BLOCKS:
                    nc.vector.tensor_reduce(
                        acc[:, j : j + 1],
                        blk[:],
                        axis=mybir.AxisListType.X,
                        op=mybir.AluOpType.add,
                        apply_absolute_value=True,
                    )
                else:
                    ab = wk_pool.tile([P, c // 2], mybir.dt.bfloat16, tag="ab")
                    nc.scalar.activation(
                        ab[:],
                        blk[:],
                        mybir.ActivationFunctionType.Abs,
                        accum_out=acc[:, j : j + 1],
                    )
                # D tiles are consumed between T blocks, matching DMA arrival.
                if j == 2:
                    emit_d_tile(0)
                if j == 6:
                    emit_d_tile(1)
            nc.sync.dma_start(out[:], acc[:])
    nc.compile()
    return nc


def _get_nc():
    if not _nc_cache:
        _nc_cache.append(_build_nc())
    return _nc_cache[0]


def _shard_inputs(yhat: np.ndarray, y: np.ndarray) -> list[dict[str, np.ndarray]]:
    yh = np.ascontiguousarray(yhat, dtype=np.float32).reshape(N_CORES, ELEMS_PER_CORE)
    yv = np.ascontiguousarray(y, dtype=np.float32).reshape(N_CORES, ELEMS_PER_CORE)
    f8 = ml_dtypes.float8_e4m3

    wm = np.zeros((P, 64), dtype=np.float32)
    for m in range(64):
        wm[m, m] = 1.0
        wm[m + 64, m] = -1.0
    wm = wm.astype(f8)

    t_cols_total = sum(T_TILE_COLS)
    in_maps = []
    for c in range(N_CORES):
        m = {"w": wm}
        yh_t = yh[c, :T_ELEMS].reshape(64, t_cols_total).astype(f8)
        yv_t = yv[c, :T_ELEMS].reshape(64, t_cols_total).astype(f8)
        off = 0
        for j, tc_ in enumerate(T_TILE_COLS):
            zt = np.empty((P, tc_), dtype=f8)
            zt[0:64] = yh_t[:, off : off + tc_]
            zt[64:128] = yv_t[:, off : off + tc_]
            m[f"zt{j}"] = zt
            off += tc_
        d_cols_total = sum(D_TILE_COLS)
        yh_d = yh[c, T_ELEMS:].reshape(P, d_cols_total)
        yv_d = yv[c, T_ELEMS:].reshape(P, d_cols_total)
        off = 0
        for i, dc in enumerate(D_TILE_COLS):
            zd = np.empty((P, 2 * dc), dtype=f8)
            zd[:, 0:dc] = yh_d[:, off : off + dc]
            zd[:, dc : 2 * dc] = yv_d[:, off : off + dc]
            m[f"zd{i}"] = zd
            off += dc
        in_maps.append(m)
    return in_maps


def kernel(yhat: np.ndarray, y: np.ndarray) -> np.ndarray:
    nc = _get_nc()
    in_maps = _shard_inputs(yhat, y)
    res = run_bass_kernel_spmd(nc, in_maps, list(range(N_CORES)))
    total = np.float64(0.0)
    for r in res.results:
        total += r["out"].astype(np.float64).sum()
    return np.asarray(total / TOTAL_ELEMS, dtype=np.float32)
